# revision 3
# baseline (speedup 1.0000x reference)
"""CNOT-ring permutation kernel for Trainium2 (Bass, 8 NeuronCores).

Problem: state [32, 2^20, 2] f32; apply ring of CNOTs CNOT(i, (i+1)%20),
i = 0..19 sequentially.  The composition is a pure index permutation:

    out[b, y, :] = in[b, x(y), :],   x(y) = (y ^ (y>>1)) ^ ((y&1) * (3<<18))

Sharding: data-parallel over batch (4 rows / core, no communication).

Per-core algorithm (verified in CoreSim):
  View each row's 2^20 amps as 1024 blocks x 1024 amps (block = 8 KiB).
  Output block bp needs input blocks X = Gray10(bp) and X^768 (even/odd
  amp split); bp and bp^512 share the same input pair, so process them
  together on one partition (exact HBM traffic).  Loads: one SWDGE
  dma_gather per tile (512 half-block indices, 4 KiB each; odd-bp blocks
  loaded half-swapped which cancels the (bp&1)<<9 term).  The remaining
  within-partition permutation on the 4096-f32 free dim is

      i_k = o_k ^ o_{k+1} (k=1..9), i11 = o1 ^ o11, i0=o0, i10=o10

  implemented as two XOR-class passes of 32 strided copies each
  (pass1 DVE: {k<-k+1, k=5..9}; pass2 ACT: {k<-k+1, k=1..4} + {11<-1}).
  Stores are plain affine HWDGE DMAs.
"""

import numpy as np

ROWS_PER_CORE = 4
N_CORES = 8
NAMP = 1 << 20            # amps per row
ROW_F32 = NAMP * 2        # f32 per row
NBLK = 1024               # blocks per row
BLK_F32 = 2048            # f32 per block (8 KiB)
HALF_F32 = 1024           # f32 per half-block (4 KiB)
TILES_PER_ROW = 4         # 128 block-pairs per tile
NF = 4096                 # f32 per partition per tile (2 blocks)


def _gray(v):
    return v ^ (v >> 1)


def make_gather_idxs():
    """int16 index tensor for dma_gather, [128, 4*32]: 4 tile planes of 32
    cols.  Tile t, plane j in {Xa, Xb, XCa, XCb}, partition p -> half-block
    index within the row's [2048, 1024 f32] view."""
    cols = []
    for t in range(TILES_PER_ROW):
        idxs = np.zeros((4, 128), np.int16)
        for p in range(128):
            bp = t * 128 + p
            X = _gray(bp)
            XC = X ^ 768
            sw = bp & 1
            idxs[0, p] = 2 * X + sw
            idxs[1, p] = 2 * X + (1 - sw)
            idxs[2, p] = 2 * XC + sw
            idxs[3, p] = 2 * XC + (1 - sw)
        flat = idxs.reshape(-1)            # order j*128 + p
        ncol = len(flat) // 16
        wrapped = flat.reshape(ncol, 16).T  # [16, ncol=32]
        cols.append(np.tile(wrapped, (8, 1)))  # replicate to 128 partitions
    return np.concatenate(cols, axis=1)    # [128, 128]


def _xor_class_pieces(tcs, nbits):
    controls = sorted({c for _, c in tcs})
    pieces = []
    for combo in range(1 << len(controls)):
        cvals = {c: (combo >> i) & 1 for i, c in enumerate(controls)}
        mask = 0
        for tb, cb in tcs:
            mask ^= cvals[cb] << tb
        pieces.append((mask, cvals))
    return pieces, controls


def build_piece_aps(AP, tile_in, tile_out, tcs, nbits=12, npart=128):
    """(dst_ap, src_ap) pairs implementing the simultaneous XOR-class map
    {target_bit ^= control_bit} on a [128, 2^nbits] f32 tile."""
    pieces, controls = _xor_class_pieces(tcs, nbits)
    cset = set(controls)
    targets = {tb: cb for tb, cb in tcs}
    free_bits = [b for b in range(nbits) if b not in cset]
    out = []
    pstride = tile_in.ap().ap[0][0]
    for mask, cvals in pieces:
        base = 0
        for c, v in cvals.items():
            base |= v << c
        src_off = base
        for tb in targets:
            if tb in cset and (mask >> tb) & 1:
                src_off ^= 1 << tb
        dims_dst = [[pstride, npart]]
        dims_src = [[pstride, npart]]
        pend = None

        def flush():
            nonlocal pend
            if pend is not None:
                dims_dst.append([1 << pend[0], 1 << pend[1]])
                dims_src.append([1 << pend[0], 1 << pend[1]])
                pend = None

        for b in sorted(free_bits, reverse=True):
            flip = (b in targets) and ((mask >> b) & 1)
            if flip:
                flush()
                dims_dst.append([1 << b, 2])
                dims_src.append([-(1 << b), 2])
                src_off += 1 << b
            else:
                if pend is not None and pend[0] == b + 1:
                    pend = [b, pend[1] + 1]
                else:
                    flush()
                    pend = [b, 1]
        flush()
        out.append((
            AP(tensor=tile_out.ap().tensor, offset=base, ap=dims_dst),
            AP(tensor=tile_in.ap().tensor, offset=src_off, ap=dims_src),
        ))
    return out


PASS1 = [(k, k + 1) for k in range(5, 10)]                # controls 6..10
PASS2 = [(k, k + 1) for k in range(1, 5)] + [(11, 1)]     # controls 1..5


def build_kernel(rows=ROWS_PER_CORE):
    """Build the per-core Bass program.  Inputs: x [rows, ROW_F32] f32,
    idx [128, 128] int16.  Output: y [rows, ROW_F32] f32."""
    import concourse.bacc as bacc
    import concourse.mybir as mybir
    from concourse.ap import AP
    from concourse.library_config import mlp

    nc = bacc.Bacc("TRN2", target_bir_lowering=False, debug=False)
    x = nc.dram_tensor("x", [rows, ROW_F32], mybir.dt.float32, kind="ExternalInput")
    idx = nc.dram_tensor("idx", [128, 128], mybir.dt.int16, kind="ExternalInput")
    y = nc.dram_tensor("y", [rows, ROW_F32], mybir.dt.float32, kind="ExternalOutput")

    ntiles = rows * TILES_PER_ROW

    with (
        nc.sbuf_tensor("tin0", [128, NF], mybir.dt.float32) as tin0,
        nc.sbuf_tensor("tin1", [128, NF], mybir.dt.float32) as tin1,
        nc.sbuf_tensor("tmid0", [128, NF], mybir.dt.float32) as tmid0,
        nc.sbuf_tensor("tmid1", [128, NF], mybir.dt.float32) as tmid1,
        nc.sbuf_tensor("tout0", [128, NF], mybir.dt.float32) as tout0,
        nc.sbuf_tensor("tout1", [128, NF], mybir.dt.float32) as tout1,
        nc.sbuf_tensor("tidx", [128, 128], mybir.dt.int16) as tidx,
        nc.semaphore("s_idx") as s_idx,
        nc.semaphore("s_in0") as s_in0,
        nc.semaphore("s_in1") as s_in1,
        nc.semaphore("s_p1") as s_p1,
        nc.semaphore("s_p2") as s_p2,
        nc.semaphore("s_out0") as s_out0,
        nc.semaphore("s_out1") as s_out1,
        nc.Block() as block,
    ):
        s_in = [s_in0, s_in1]
        s_out = [s_out0, s_out1]
        tin = [tin0, tin1]
        tmid = [tmid0, tmid1]
        tout = [tout0, tout1]

        # precompute per-buffer piece AP lists
        p1_aps = [build_piece_aps(AP, tin[b], tmid[b], PASS1) for b in range(2)]
        p2_aps = [build_piece_aps(AP, tmid[b], tout[b], PASS2) for b in range(2)]

        xv = x.rearrange("r (n e) -> r n e", e=HALF_F32)   # [rows, 2048, 1024]
        yv = y.rearrange("r (n e) -> r n e", e=BLK_F32)    # [rows, 1024, 2048]

        @block.gpsimd
        def _(g):
            g.load_library(mlp)
            g.dma_start(tidx[:, :], idx[:, :]).then_inc(s_idx, 16)
            g.wait_ge(s_idx, 16)
            for i in range(ntiles):
                r, t = divmod(i, TILES_PER_ROW)
                b = i % 2
                if i >= 2:
                    g.wait_ge(s_p1, i - 1)
                g.dma_gather(
                    tin[b][:, :].rearrange("p (j e) -> p j e", e=HALF_F32),
                    xv[r],
                    tidx[:, t * 32:(t + 1) * 32],
                    512, 512, HALF_F32,
                ).then_inc(s_in[b], 16)

        @block.vector
        def _(v):
            for i in range(ntiles):
                b = i % 2
                v.wait_ge(s_in[b], 16 * (i // 2 + 1))
                if i >= 2:
                    v.wait_ge(s_p2, i - 1)
                aps = p1_aps[b]
                for n, (dst, src) in enumerate(aps):
                    ins = v.tensor_copy(dst, src)
                    if n == len(aps) - 1:
                        ins.then_inc(s_p1, 1)

        @block.scalar
        def _(s):
            for i in range(ntiles):
                b = i % 2
                s.wait_ge(s_p1, i + 1)
                if i >= 2:
                    s.wait_ge(s_out[b], 32 * (i // 2))
                aps = p2_aps[b]
                for n, (dst, src) in enumerate(aps):
                    ins = s.copy(dst, src)
                    if n == len(aps) - 1:
                        ins.then_inc(s_p2, 1)

        @block.sync
        def _(sy):
            for i in range(ntiles):
                r, t = divmod(i, TILES_PER_ROW)
                b = i % 2
                sy.wait_ge(s_p2, i + 1)
                sy.dma_start(
                    yv[r, t * 128:(t + 1) * 128, :], tout[b][:, 0:BLK_F32]
                ).then_inc(s_out[b], 16)
                sy.dma_start(
                    yv[r, 512 + t * 128: 512 + (t + 1) * 128, :], tout[b][:, BLK_F32:NF]
                ).then_inc(s_out[b], 16)
            for b in range(2):
                n_b = ntiles // 2 + (ntiles % 2 if b == 0 else 0)
                sy.wait_ge(s_out[b], 32 * n_b)

    nc.compile()
    return nc


_IDX = None
_NC = None


def kernel(state: np.ndarray) -> np.ndarray:
    """Full-input entry point: state [32, 2^20, 2] f32 -> same shape."""
    global _IDX, _NC
    from concourse.bass_utils import run_bass_kernel_spmd

    assert state.shape == (32, NAMP, 2) and state.dtype == np.float32
    if _IDX is None:
        _IDX = make_gather_idxs()
    if _NC is None:
        _NC = build_kernel(ROWS_PER_CORE)

    in_maps = []
    for c in range(N_CORES):
        xs = np.ascontiguousarray(
            state[c * ROWS_PER_CORE:(c + 1) * ROWS_PER_CORE]
        ).reshape(ROWS_PER_CORE, ROW_F32)
        in_maps.append({"x": xs, "idx": _IDX})

    res = run_bass_kernel_spmd(_NC, in_maps, core_ids=list(range(N_CORES)))
    out = np.empty((32, NAMP, 2), np.float32)
    for c in range(N_CORES):
        out[c * ROWS_PER_CORE:(c + 1) * ROWS_PER_CORE] = res.results[c]["y"].reshape(
            ROWS_PER_CORE, NAMP, 2
        )
    return out


if __name__ == "__main__":
    rng = np.random.default_rng(0)
    state = rng.standard_normal((32, NAMP, 2)).astype(np.float32)
    out = kernel(state)
    yy = np.arange(NAMP)
    xx = (yy ^ (yy >> 1)) ^ ((yy & 1) * (3 << 18))
    exp = state[:, xx, :]
    print("match:", np.array_equal(out, exp))


# revision 7
# speedup vs baseline: 1.0345x; 1.0345x over previous
"""CNOT-ring permutation kernel for Trainium2 (Bass, 8 NeuronCores).

Problem: state [32, 2^20, 2] f32; apply ring of CNOTs CNOT(i, (i+1)%20),
i = 0..19 sequentially.  The composition is a pure index permutation:

    out[b, y, :] = in[b, x(y), :],   x(y) = (y ^ (y>>1)) ^ ((y&1) * (3<<18))

Sharding: data-parallel over batch (4 rows / core, no communication).

Per-core algorithm (verified in CoreSim):
  View each row's 2^20 amps as 1024 blocks x 1024 amps (block = 8 KiB).
  Output block bp needs input blocks X = Gray10(bp) and X^768 (even/odd
  amp split); bp and bp^512 share the same input pair, so process them
  together on one partition (exact HBM traffic).  Loads: one SWDGE
  dma_gather per tile (512 half-block indices, 4 KiB each; odd-bp blocks
  loaded half-swapped which cancels the (bp&1)<<9 term).  The remaining
  within-partition permutation on the 4096-f32 free dim is

      i_k = o_k ^ o_{k+1} (k=1..9), i11 = o1 ^ o11, i0=o0, i10=o10

  implemented as two XOR-class passes of 32 strided copies each
  (pass1 DVE: {k<-k+1, k=5..9}; pass2 ACT: {k<-k+1, k=1..4} + {11<-1}).
  Stores are plain affine HWDGE DMAs.
"""

import numpy as np

ROWS_PER_CORE = 4
N_CORES = 8
NAMP = 1 << 20            # amps per row
ROW_F32 = NAMP * 2        # f32 per row
NBLK = 1024               # blocks per row
BLK_F32 = 2048            # f32 per block (8 KiB)
HALF_F32 = 1024           # f32 per half-block (4 KiB)
TILES_PER_ROW = 4         # 128 block-pairs per tile
NF = 4096                 # f32 per partition per tile (2 blocks)


def _gray(v):
    return v ^ (v >> 1)


def make_gather_idxs():
    """int16 index tensor for dma_gather, [128, 4*32]: 4 tile planes of 32
    cols.  Tile t, plane j in {Xa, Xb, XCa, XCb}, partition p -> half-block
    index within the row's [2048, 1024 f32] view."""
    cols = []
    for t in range(TILES_PER_ROW):
        idxs = np.zeros((4, 128), np.int16)
        for p in range(128):
            bp = t * 128 + p
            X = _gray(bp)
            XC = X ^ 768
            sw = bp & 1
            idxs[0, p] = 2 * X + sw
            idxs[1, p] = 2 * X + (1 - sw)
            idxs[2, p] = 2 * XC + sw
            idxs[3, p] = 2 * XC + (1 - sw)
        flat = idxs.reshape(-1)            # order j*128 + p
        ncol = len(flat) // 16
        wrapped = flat.reshape(ncol, 16).T  # [16, ncol=32]
        cols.append(np.tile(wrapped, (8, 1)))  # replicate to 128 partitions
    return np.concatenate(cols, axis=1)    # [128, 128]


def _xor_class_pieces(tcs, nbits):
    controls = sorted({c for _, c in tcs})
    pieces = []
    for combo in range(1 << len(controls)):
        cvals = {c: (combo >> i) & 1 for i, c in enumerate(controls)}
        mask = 0
        for tb, cb in tcs:
            mask ^= cvals[cb] << tb
        pieces.append((mask, cvals))
    return pieces, controls


def build_piece_aps(AP, tile_in, tile_out, tcs, nbits=12, npart=128):
    """(dst_ap, src_ap) pairs implementing the simultaneous XOR-class map
    {target_bit ^= control_bit} on a [128, 2^nbits] f32 tile."""
    pieces, controls = _xor_class_pieces(tcs, nbits)
    cset = set(controls)
    targets = {tb: cb for tb, cb in tcs}
    free_bits = [b for b in range(nbits) if b not in cset]
    out = []
    pstride = tile_in.ap().ap[0][0]
    for mask, cvals in pieces:
        base = 0
        for c, v in cvals.items():
            base |= v << c
        src_off = base
        for tb in targets:
            if tb in cset and (mask >> tb) & 1:
                src_off ^= 1 << tb
        dims_dst = [[pstride, npart]]
        dims_src = [[pstride, npart]]
        pend = None

        def flush():
            nonlocal pend
            if pend is not None:
                dims_dst.append([1 << pend[0], 1 << pend[1]])
                dims_src.append([1 << pend[0], 1 << pend[1]])
                pend = None

        for b in sorted(free_bits, reverse=True):
            flip = (b in targets) and ((mask >> b) & 1)
            if flip:
                flush()
                dims_dst.append([1 << b, 2])
                dims_src.append([-(1 << b), 2])
                src_off += 1 << b
            else:
                if pend is not None and pend[0] == b + 1:
                    pend = [b, pend[1] + 1]
                else:
                    flush()
                    pend = [b, 1]
        flush()
        # move the largest identity dim innermost: segment count = product of
        # outer dim counts, and inner-dim stride need not be 1 for 1x ops
        free_dims = list(zip(dims_dst[1:], dims_src[1:]))
        big = max(range(len(free_dims)), key=lambda i: free_dims[i][0][1])
        free_dims.append(free_dims.pop(big))
        dims_dst = dims_dst[:1] + [d for d, _ in free_dims]
        dims_src = dims_src[:1] + [s for _, s in free_dims]
        out.append((
            AP(tensor=tile_out.ap().tensor, offset=base, ap=dims_dst),
            AP(tensor=tile_in.ap().tensor, offset=src_off, ap=dims_src),
        ))
    return out


PASS1 = [(k, k + 1) for k in range(5, 10)]                # controls 6..10
PASS2 = [(k, k + 1) for k in range(1, 5)] + [(11, 1)]     # controls 1..5
DVE_P2 = 4   # pass2 pieces done by DVE (rest on ACT)


def build_kernel(rows=ROWS_PER_CORE):
    """Build the per-core Bass program.  Inputs: x [rows, ROW_F32] f32,
    idx [128, 128] int16.  Output: y [rows, ROW_F32] f32."""
    import concourse.bacc as bacc
    import concourse.mybir as mybir
    from concourse.ap import AP
    from concourse.library_config import mlp

    nc = bacc.Bacc("TRN2", target_bir_lowering=False, debug=False)
    x = nc.dram_tensor("x", [rows, ROW_F32], mybir.dt.float32, kind="ExternalInput")
    idx = nc.dram_tensor("idx", [128, 128], mybir.dt.int16, kind="ExternalInput")
    y = nc.dram_tensor("y", [rows, ROW_F32], mybir.dt.float32, kind="ExternalOutput")

    ntiles = rows * TILES_PER_ROW

    with (
        nc.sbuf_tensor("tin0", [128, NF], mybir.dt.float32) as tin0,
        nc.sbuf_tensor("tin1", [128, NF], mybir.dt.float32) as tin1,
        nc.sbuf_tensor("tmid0", [128, NF], mybir.dt.float32) as tmid0,
        nc.sbuf_tensor("tmid1", [128, NF], mybir.dt.float32) as tmid1,
        nc.sbuf_tensor("tout0", [128, NF], mybir.dt.float32) as tout0,
        nc.sbuf_tensor("tout1", [128, NF], mybir.dt.float32) as tout1,
        nc.sbuf_tensor("tidx", [128, 128], mybir.dt.int16) as tidx,
        nc.semaphore("s_idx") as s_idx,
        nc.semaphore("s_in0") as s_in0,
        nc.semaphore("s_in1") as s_in1,
        nc.semaphore("s_p1") as s_p1,
        nc.semaphore("s_p2") as s_p2,
        nc.semaphore("s_p2v") as s_p2v,
        nc.semaphore("s_out0") as s_out0,
        nc.semaphore("s_out1") as s_out1,
        nc.Block() as block,
    ):
        s_in = [s_in0, s_in1]
        s_out = [s_out0, s_out1]
        tin = [tin0, tin1]
        tmid = [tmid0, tmid1]
        tout = [tout0, tout1]

        # precompute per-buffer piece AP lists
        p1_aps = [build_piece_aps(AP, tin[b], tmid[b], PASS1) for b in range(2)]
        p2_aps = [build_piece_aps(AP, tmid[b], tout[b], PASS2) for b in range(2)]

        xv = x.rearrange("r (n e) -> r n e", e=HALF_F32)   # [rows, 2048, 1024]
        yv = y.rearrange("r (n e) -> r n e", e=BLK_F32)    # [rows, 1024, 2048]

        @block.gpsimd
        def _(g):
            g.load_library(mlp)
            g.dma_start(tidx[:, :], idx[:, :]).then_inc(s_idx, 16)
            g.wait_ge(s_idx, 16)
            for i in range(ntiles):
                r, t = divmod(i, TILES_PER_ROW)
                b = i % 2
                if i >= 2:
                    g.wait_ge(s_p1, i - 1)
                g.dma_gather(
                    tin[b][:, :].rearrange("p (j e) -> p j e", e=HALF_F32),
                    xv[r],
                    tidx[:, t * 32:(t + 1) * 32],
                    512, 512, HALF_F32,
                ).then_inc(s_in[b], 16)

        @block.vector
        def _(v):
            for i in range(ntiles):
                b = i % 2
                v.wait_ge(s_in[b], 16 * (i // 2 + 1))
                if i >= 2:
                    v.wait_ge(s_p2, i - 1)          # ACT done reading tmid[b]
                aps = p1_aps[b]
                for n, (dst, src) in enumerate(aps):
                    ins = v.tensor_copy(dst, src)
                    if n == len(aps) - 1:
                        ins.then_inc(s_p1, 1)
                # DVE's share of pass2 (reads tmid[b] it just wrote; the
                # self-wait on s_p1 orders it after the pass1 datapath)
                if DVE_P2:
                    v.wait_ge(s_p1, i + 1)
                    if i >= 2:
                        v.wait_ge(s_out[b], 32 * (i // 2))
                    aps2 = p2_aps[b][:DVE_P2]
                    for n, (dst, src) in enumerate(aps2):
                        ins = v.tensor_copy(dst, src)
                        if n == len(aps2) - 1:
                            ins.then_inc(s_p2v, 1)

        @block.scalar
        def _(s):
            for i in range(ntiles):
                b = i % 2
                s.wait_ge(s_p1, i + 1)
                if i >= 2:
                    s.wait_ge(s_out[b], 32 * (i // 2))
                aps = p2_aps[b][DVE_P2:]
                for n, (dst, src) in enumerate(aps):
                    ins = s.copy(dst, src)
                    if n == len(aps) - 1:
                        ins.then_inc(s_p2, 1)

        @block.sync
        def _(sy):
            for i in range(ntiles):
                r, t = divmod(i, TILES_PER_ROW)
                b = i % 2
                sy.wait_ge(s_p2, i + 1)
                if DVE_P2:
                    sy.wait_ge(s_p2v, i + 1)
                sy.dma_start(
                    yv[r, t * 128:(t + 1) * 128, :], tout[b][:, 0:BLK_F32]
                ).then_inc(s_out[b], 16)
                sy.dma_start(
                    yv[r, 512 + t * 128: 512 + (t + 1) * 128, :], tout[b][:, BLK_F32:NF]
                ).then_inc(s_out[b], 16)
            for b in range(2):
                n_b = ntiles // 2 + (ntiles % 2 if b == 0 else 0)
                sy.wait_ge(s_out[b], 32 * n_b)

    nc.compile()
    return nc


_IDX = None
_NC = None


def kernel(state: np.ndarray) -> np.ndarray:
    """Full-input entry point: state [32, 2^20, 2] f32 -> same shape."""
    global _IDX, _NC
    from concourse.bass_utils import run_bass_kernel_spmd

    assert state.shape == (32, NAMP, 2) and state.dtype == np.float32
    if _IDX is None:
        _IDX = make_gather_idxs()
    if _NC is None:
        _NC = build_kernel(ROWS_PER_CORE)

    in_maps = []
    for c in range(N_CORES):
        xs = np.ascontiguousarray(
            state[c * ROWS_PER_CORE:(c + 1) * ROWS_PER_CORE]
        ).reshape(ROWS_PER_CORE, ROW_F32)
        in_maps.append({"x": xs, "idx": _IDX})

    res = run_bass_kernel_spmd(_NC, in_maps, core_ids=list(range(N_CORES)))
    out = np.empty((32, NAMP, 2), np.float32)
    for c in range(N_CORES):
        out[c * ROWS_PER_CORE:(c + 1) * ROWS_PER_CORE] = res.results[c]["y"].reshape(
            ROWS_PER_CORE, NAMP, 2
        )
    return out


if __name__ == "__main__":
    rng = np.random.default_rng(0)
    state = rng.standard_normal((32, NAMP, 2)).astype(np.float32)
    out = kernel(state)
    yy = np.arange(NAMP)
    xx = (yy ^ (yy >> 1)) ^ ((yy & 1) * (3 << 18))
    exp = state[:, xx, :]
    print("match:", np.array_equal(out, exp))
